# revision 5
# baseline (speedup 1.0000x reference)
"""Trainium2 Bass kernel for nn_DifferentiableAggregation_avg (segment reduce).

Strategy (per sharding hint): partition the 262144 output segments across the
8 cores (32768 segments each, disjoint). On the host, rows are sorted by
segment id and laid out into a per-core padded layout where each tile of 128
segments (one per SBUF partition) gets a uniform per-tile slot capacity
(= max row count over the tile's segments, tight because segments are sorted
by count). The device then streams the layout and performs all of the
reference's arithmetic: per-row 3-class max, per-segment sums of logit0,
logit1+logit2, row-max, label==4 / label==1 counts, and the final
sigmoid combine. Each core owns a disjoint slice of segments, so no
cross-core reduction is needed.
"""
import sys

sys.path.insert(0, "/opt/trn_rl_repo")

import numpy as np

NSEG = 262144
NCORES = 8
SEGS_PER_CORE = NSEG // NCORES  # 32768
PART = 128
T = SEGS_PER_CORE // PART  # 256 tiles per core


def _split_multiwaits(nc, max_waits=1):
    """walrus codegen in this container only encodes one sync wait on ctrl
    ops (Drain): hoist extra waits onto single-wait no-ops just before."""
    import concourse.mybir as mybir

    n = 0
    for f in nc.m.functions:
        for bb in f.blocks:
            new_insts = []
            for ins in bb.instructions:
                si = getattr(ins, "sync_info", None)
                if si is not None and si.on_wait and len(si.on_wait) > max_waits:
                    waits = list(si.on_wait)
                    for w in waits[:-max_waits]:
                        nop = mybir.InstNoOp(
                            name=f"I-splitwait-{n}",
                            engine=ins.engine,
                            sync_info=mybir.SyncInfo(on_wait=[w], on_update=[]),
                        )
                        n += 1
                        new_insts.append(nop)
                    ins.sync_info = mybir.SyncInfo(
                        on_wait=waits[-max_waits:], on_update=list(si.on_update)
                    )
                new_insts.append(ins)
            bb.instructions = new_insts
    return n


def build_nc(cap1, cap2, ntiles, split=True):
    """Build the per-core Bass program. cap1/cap2: per-tile slot capacities
    (same schedule for all cores). Inputs:
      L  : flat f32 [sum(cap1)*128*3]   padded logit rows, tile-major
      B  : flat f32 [sum(cap2)*128]     padded label rows, tile-major
      C  : f32 [128, ntiles]            true per-segment row counts
    Output:
      out: f32 [128, 2*ntiles]          (j0, j1) interleaved per tile column
    """
    import concourse.bass as bass
    import concourse.mybir as mybir
    from concourse.tile import TileContext

    f32 = mybir.dt.float32
    Alu = mybir.AluOpType
    Act = mybir.ActivationFunctionType

    off1 = np.concatenate([[0], np.cumsum(cap1)]).astype(np.int64)
    off2 = np.concatenate([[0], np.cumsum(cap2)]).astype(np.int64)
    tot1 = int(off1[-1])
    tot2 = int(off2[-1])

    nc = bass.Bass("TRN2")
    L = nc.dram_tensor("L", [tot1 * PART * 3], f32, kind="ExternalInput")
    B = nc.dram_tensor("B", [tot2 * PART], f32, kind="ExternalInput")
    C = nc.dram_tensor("C", [PART, ntiles], f32, kind="ExternalInput")
    O = nc.dram_tensor("out", [PART, 2 * ntiles], f32, kind="ExternalOutput")

    with TileContext(nc) as tc:
        with tc.tile_pool(name="acc", bufs=1) as acc, \
             tc.tile_pool(name="work", bufs=4) as work, \
             tc.tile_pool(name="scr", bufs=3) as scrp:
            s0c = acc.tile([PART, ntiles], f32, tag="s0c", name="s0c")
            s12c = acc.tile([PART, ntiles], f32, tag="s12c", name="s12c")
            smaxc = acc.tile([PART, ntiles], f32, tag="smaxc", name="smaxc")
            c4c = acc.tile([PART, ntiles], f32, tag="c4c", name="c4c")
            c1c = acc.tile([PART, ntiles], f32, tag="c1c", name="c1c")
            ctsb = acc.tile([PART, ntiles], f32, tag="ctsb", name="ctsb")
            outsb = acc.tile([PART, 2 * ntiles], f32, tag="outsb", name="outsb")

            nc.sync.dma_start(ctsb, C[:, :])

            for t in range(ntiles):
                w1 = int(cap1[t])
                w2 = int(cap2[t])
                a1 = int(off1[t]) * PART * 3
                a2 = int(off2[t]) * PART
                Lt = work.tile([PART, w1 * 3], f32, tag="Lt", name=f"Lt{t}")
                Bt = work.tile([PART, w2], f32, tag="Bt", name=f"Bt{t}")
                nc.sync.dma_start(
                    Lt, L[a1 : a1 + PART * w1 * 3].rearrange("(p x) -> p x", p=PART)
                )
                nc.sync.dma_start(
                    Bt, B[a2 : a2 + PART * w2].rearrange("(p x) -> p x", p=PART)
                )
                L3 = Lt.rearrange("p (s c) -> p s c", c=3)
                sA = scrp.tile([PART, w1], f32, tag="sA", name=f"sA{t}")
                sB = scrp.tile([PART, w1, 2], f32, tag="sB", name=f"sB{t}")
                nc.scalar.activation(
                    sA, L3[:, :, 0], Act.Copy, accum_out=s0c[:, t : t + 1]
                )
                nc.scalar.activation(
                    sB, L3[:, :, 1:3], Act.Copy, accum_out=s12c[:, t : t + 1]
                )
                m01 = scrp.tile([PART, w1], f32, tag="m01", name=f"m01{t}")
                nc.vector.tensor_tensor(m01, L3[:, :, 0], L3[:, :, 1], Alu.max)
                sM = scrp.tile([PART, w1], f32, tag="sM", name=f"sM{t}")
                nc.vector.tensor_tensor(sM, m01, L3[:, :, 2], Alu.max)
                nc.vector.tensor_reduce(
                    smaxc[:, t : t + 1], sM, mybir.AxisListType.X, Alu.add
                )
                s4 = scrp.tile([PART, w2], f32, tag="s4", name=f"s4{t}")
                nc.vector.tensor_scalar(
                    s4, Bt, 4.0, None, Alu.is_equal, op1=Alu.add, accum_out=c4c[:, t : t + 1]
                )
                s1 = scrp.tile([PART, w2], f32, tag="s1", name=f"s1{t}")
                nc.vector.tensor_scalar(
                    s1, Bt, 1.0, None, Alu.is_equal, op1=Alu.add, accum_out=c1c[:, t : t + 1]
                )

            # final combine on [128, ntiles]
            fin = acc
            safe = fin.tile([PART, ntiles], f32, tag="safe", name="safe")
            nc.vector.tensor_scalar_max(safe, ctsb, 1.0)
            inv = fin.tile([PART, ntiles], f32, tag="inv", name="inv")
            nc.vector.reciprocal(inv, safe)
            avg = fin.tile([PART, ntiles], f32, tag="avg", name="avg")
            nc.vector.tensor_tensor(avg, smaxc, inv, Alu.mult)
            small = fin.tile([PART, ntiles], f32, tag="small", name="small")
            nc.vector.tensor_scalar(small, ctsb, 6.0, None, Alu.is_lt)
            c4m = fin.tile([PART, ntiles], f32, tag="c4m", name="c4m")
            nc.vector.tensor_tensor(c4m, c4c, small, Alu.mult)
            c1m = fin.tile([PART, ntiles], f32, tag="c1m", name="c1m")
            nc.vector.tensor_tensor(c1m, c1c, small, Alu.mult)
            u0 = fin.tile([PART, ntiles], f32, tag="u0", name="u0")
            nc.vector.scalar_tensor_tensor(
                u0, c1m, -5.0, avg, op0=Alu.add, op1=Alu.mult
            )
            u1 = fin.tile([PART, ntiles], f32, tag="u1", name="u1")
            nc.vector.scalar_tensor_tensor(
                u1, c4m, -1.0, avg, op0=Alu.add, op1=Alu.mult
            )
            a0 = fin.tile([PART, ntiles], f32, tag="a0", name="a0")
            nc.vector.tensor_tensor(a0, s0c, u0, Alu.add)
            a1t = fin.tile([PART, ntiles], f32, tag="a1t", name="a1t")
            nc.vector.tensor_tensor(a1t, s12c, u1, Alu.add)
            OS = outsb.rearrange("p (t c) -> p t c", c=2)
            nc.scalar.activation(OS[:, :, 0], a0, Act.Sigmoid, scale=10.0)
            nc.scalar.activation(OS[:, :, 1], a1t, Act.Sigmoid, scale=10.0)
            nc.sync.dma_start(O[:, :], outsb)

    if split:
        _split_multiwaits(nc)
    return nc


def prepare(sub_logits, original_indices, full_sub_labels, full_original_indices):
    """Host-side shard/sort/pad. Returns (in_maps, seg_order, cap1, cap2)."""
    sub_logits = np.ascontiguousarray(np.asarray(sub_logits, dtype=np.float32))
    seg = np.asarray(original_indices).astype(np.int32)
    lab = np.asarray(full_sub_labels).astype(np.float32)
    fseg = np.asarray(full_original_indices).astype(np.int32)
    n = seg.shape[0]

    c1 = np.bincount(seg, minlength=NSEG).astype(np.int64)
    c2 = np.bincount(fseg, minlength=NSEG).astype(np.int64)

    # per-core segment ordering by (count1, count2)
    seg_order = np.empty(NSEG, np.int32)
    rank = np.empty(NSEG, np.int32)
    for d in range(NCORES):
        sl = slice(d * SEGS_PER_CORE, (d + 1) * SEGS_PER_CORE)
        o = np.lexsort((c2[sl], c1[sl])).astype(np.int32)
        ids = (d * SEGS_PER_CORE + o).astype(np.int32)
        seg_order[sl] = ids
        rank[ids] = np.arange(SEGS_PER_CORE, dtype=np.int32)

    c1o = c1[seg_order].reshape(NCORES, T, PART)
    c2o = c2[seg_order].reshape(NCORES, T, PART)
    cap1 = c1o.max(axis=(0, 2))
    cap2 = c2o.max(axis=(0, 2))
    cap1 = np.maximum((cap1 + 1) // 2 * 2, 2).astype(np.int64)
    cap2 = np.maximum((cap2 + 1) // 2 * 2, 2).astype(np.int64)
    off1 = np.concatenate([[0], np.cumsum(cap1)])
    off2 = np.concatenate([[0], np.cumsum(cap2)])
    tot1 = int(off1[-1])
    tot2 = int(off2[-1])

    def scatter(values, segv, counts, caps_off, tot, width):
        # values: [n, width] f32; returns [NCORES, tot*128*width]
        order = np.argsort(segv, kind="stable")
        sseg = segv[order]
        starts = np.concatenate([[0], np.cumsum(counts)]).astype(np.int64)
        k = np.arange(n, dtype=np.int64) - starts[sseg]
        r = rank[sseg].astype(np.int64)
        tt = r >> 7
        p = r & 127
        slot = caps_off[0][tt] * PART + p * caps_off[1][tt] + k
        core = (sseg >> 15).astype(np.int64)
        out = np.zeros((NCORES, tot * PART * width), np.float32)
        flat_idx = core * (tot * PART * width) + slot * width
        big = out.reshape(-1)
        vals = values[order]
        if width == 1:
            big[flat_idx] = vals[:, 0]
        else:
            for ch in range(width):
                big[flat_idx + ch] = vals[:, ch]
        return out

    Lpad = scatter(sub_logits, seg, c1, (off1, cap1), tot1, 3)
    Bpad = scatter(lab.reshape(-1, 1), fseg, c2, (off2, cap2), tot2, 1)

    cts = c1o.transpose(0, 2, 1).astype(np.float32)  # [NCORES, 128, T]

    in_maps = [
        {"L": Lpad[d], "B": Bpad[d], "C": np.ascontiguousarray(cts[d])}
        for d in range(NCORES)
    ]
    return in_maps, seg_order, cap1, cap2


def unshard(results, seg_order):
    out = np.empty((NSEG, 2), np.float32)
    for d in range(NCORES):
        o = results[d]["out"]  # [128, 2T]
        j = o.reshape(PART, T, 2).transpose(1, 0, 2).reshape(SEGS_PER_CORE, 2)
        out[seg_order[d * SEGS_PER_CORE : (d + 1) * SEGS_PER_CORE]] = j
    return out


_CACHE = {}


def kernel(sub_logits, original_indices, full_sub_labels, full_original_indices):
    from concourse.bass_utils import run_bass_kernel_spmd

    in_maps, seg_order, cap1, cap2 = prepare(
        sub_logits, original_indices, full_sub_labels, full_original_indices
    )
    key = (tuple(cap1.tolist()), tuple(cap2.tolist()))
    nc = _CACHE.get(key)
    if nc is None:
        nc = build_nc(cap1, cap2, T)
        _CACHE[key] = nc
    res = run_bass_kernel_spmd(nc, in_maps, core_ids=list(range(NCORES)))
    return unshard(res.results, seg_order)
